# revision 1
# baseline (speedup 1.0000x reference)
"""AI4DEM contact-force kernel for 8 TRN2 NeuronCores.

Physics (from the reference): two particle layers on a 128^3 grid; for each
layer n, accumulate spring-damper contact forces from both layers over a
5x5x5 neighborhood of rolls, then integrate velocities.  Because
cell_size == particle_size == 0.1 and particle jitter < cell_size, any
offset with |shift| >= 2 in some axis can never produce a contact
(dist > PS provably), so the 125-point stencil reduces exactly to 3x3x3.
Roll wrap-around contributions are likewise provably zero (positions
differ by ~12.7), so the stencil is a pure local halo-1 stencil with
far-value sentinels at the global boundary.

Distribution: shard z (first spatial axis) across the 8 cores, 16 planes
each.  Layer-1 forces depend on layer-0's *updated* velocities, so each
core takes a halo of 2 input planes per side (inputs host-padded with
sentinel planes so all cores run an identical program) and no inter-core
communication is needed at all.

Layout on core: partition dim = y (128 rows), free dims = (z-chunk, x).
y-shifted stencil reads are materialized as 3 y-offset DMA loads from the
host-padded (y=130) arrays; z and x shifts are free-dim window offsets.

Precision split: the geometry path (position deltas, dist^2, contact gate,
spring term) is fp32 so the contact gate agrees with the reference to ~1
ulp; the damping path (velocity deltas, relative-velocity dot) and the
force direction products run in bf16 (DVE 2x mode), with accumulation in
fp32.  1/dist and ETA/dist^2 come from Exp(-0.5*Ln(s)) / Exp(-Ln(s)+lnETA)
on the Scalar engine (single activation-table set; DVE reciprocal is ~6x
slower).

Engine split (measured): GpSimd shares SBUF ports with DVE, so 2-input
tensor ops run there at a combined throughput *below* DVE alone — all
tensor-tensor work stays on DVE; the Scalar engine runs concurrently at
full speed and takes every single-input op (squares, bf16 casts, ln/exp,
the spring affine).  GpSimd only does memsets.
"""

import math
import sys

import numpy as np

sys.path.insert(0, "/opt/trn_rl_repo")

D = 128
CELL = 0.1
PS = 0.1
KN = 6.0e6
_ALPHA = -math.log(0.5) / math.pi
_GAMMA = _ALPHA / math.sqrt(_ALPHA**2 + 1.0)
PM = 4.0 / 3.0 * 3.1415 * CELL**3 * 2700.0
ETA = 2.0 * _GAMMA * math.sqrt(KN * PM)
DT = 1e-4
SENT = 1.0e3      # far-value sentinel for positions at global boundaries
NCORES = 8
ZP = D // NCORES  # 16 output planes per core
NZIN = ZP + 4     # input planes per core (halo 2 each side)
NY = D + 2        # host-padded y extent
NX = D + 2        # host-padded x extent

BLO_HI = 1.5 * PS
BHI_TH = D * CELL - 0.5 * PS - CELL
GRAV = -9.8 * PM

ZC_MAX = 6
ZCP2 = ZC_MAX + 2

SHIFTS = [(a, b, c) for a in (-1, 0, 1) for b in (-1, 0, 1) for c in (-1, 0, 1)]

POS_NAMES = ["xg", "yg", "zg"]
VEL_NAMES = ["vx", "vy", "vz"]

_compiled = None


def _build():
    from contextlib import ExitStack
    from concourse import bacc, tile, mybir

    f32 = mybir.dt.float32
    bf16 = mybir.dt.bfloat16
    A = mybir.AluOpType
    AF = mybir.ActivationFunctionType

    nc = bacc.Bacc("TRN2", target_bir_lowering=False, debug=False)

    # All ACT funcs used here (Square, Copy, Identity, Ln, Exp) live in the
    # "natural_log_exp_and_others" table set, but the default first-match
    # table choice pairs Exp with set 0 and Ln with set 5, inserting ~640
    # table reloads (~2.7us each).  Blank every other set (the cached dict
    # is shared, and set *indices* are positional, so contents must be
    # emptied rather than removed) so one table load serves the kernel.
    from concourse import hw_specs
    tabs = hw_specs.get_activation_tables(nc.m.arch)
    for k in tabs:
        if k != "natural_log_exp_and_others":
            tabs[k] = set()

    ext = {}
    for f in POS_NAMES + VEL_NAMES + ["mk"]:
        ext[f] = nc.dram_tensor(f, [2, NZIN, NY, NX], f32, kind="ExternalInput").ap()
    extb = {}
    for f in VEL_NAMES:
        extb[f] = nc.dram_tensor(f + "b", [2, NZIN, NY, NX], bf16,
                                 kind="ExternalInput").ap()
    out_ext = nc.dram_tensor("out", [6, ZP, D, D], f32, kind="ExternalOutput").ap()
    # layer-0 updated velocities (bf16: damping-only consumer) for phase 1
    scr = nc.dram_tensor("v0s", [3, ZP + 2, NY, NX], bf16).ap()

    with tile.TileContext(nc) as tc:
        with ExitStack() as ctx:
            pool = ctx.enter_context(tc.tile_pool(name="sbuf", bufs=1))

            def mktile(name, shape, dtp):
                return pool.tile(shape, dtp, name=name, tag=name)

            # n-side aligned: positions+velocities fp32 (geometry / update)
            nt = [mktile(f"nt_{f}", [D, ZCP2, NX], f32) for f in range(6)]
            # n-side aligned velocities bf16 (damping delta operand)
            ntb = [mktile(f"ntb_{f}", [D, ZCP2, NX], bf16) for f in range(3)]
            # m-side positions fp32, velocities bf16; 3 y-variants each
            mp = {(f, v): mktile(f"mp_{f}_{v}", [D, ZCP2, NX], f32)
                  for f in range(3) for v in (-1, 0, 1)}
            mv = {(f, v): mktile(f"mv_{f}_{v}", [D, ZCP2, NX], bf16)
                  for f in range(3) for v in (-1, 0, 1)}
            mk = mktile("mk", [D, ZC_MAX, NX], f32)

            inter = {}
            for tg in ["dX", "dY", "dZ", "QA", "QB", "QC", "S", "LN", "R"]:
                inter[tg] = mktile(tg, [D, ZC_MAX, D], f32)
            for tg in ["dXb", "dYb", "dZb", "R2", "VA", "VB", "VC",
                       "P1", "P2", "C", "TX", "TY", "TZ", "Aa", "G", "P3"]:
                inter[tg] = mktile(tg, [D, ZC_MAX, D], bf16)
            FX = mktile("FX", [D, ZC_MAX, D], f32)
            FY = mktile("FY", [D, ZC_MAX, D], f32)
            FZ = mktile("FZ", [D, ZC_MAX, D], f32)
            VN = [mktile(f"VN{i}", [D, ZC_MAX, NX], f32) for i in range(3)]
            VNB = [mktile(f"VNB{i}", [D, ZC_MAX, NX], bf16) for i in range(3)]

            b_eps = mktile("b_eps", [D, 1], f32)
            b_lneta = mktile("b_lneta", [D, 1], f32)
            b_kn = mktile("b_kn", [D, 1], f32)
            zt = mktile("zt", [D, NX], bf16)
            nc.vector.memset(b_eps[:], 1e-8)
            nc.vector.memset(b_lneta[:], math.log(ETA))
            nc.vector.memset(b_kn[:], KN)
            nc.vector.memset(zt[:], 0.0)

            for f in range(3):
                nc.sync.dma_start(scr[f, :, 0, :], zt[0:ZP + 2, :])
                nc.sync.dma_start(scr[f, :, NY - 1, :], zt[0:ZP + 2, :])

            def load(tile_t, src_ap):
                nc.sync.dma_start(tile_t, src_ap.rearrange("z y x -> y z x"))

            def emit_combo(zc, sh, npos, nvelb, mpos, mvelb):
                shz, shy, shx = sh
                v = -shy
                z0, x0 = 1 - shz, 1 - shx
                msl = (slice(None), slice(z0, z0 + zc), slice(x0, x0 + D))
                nsl = (slice(None), slice(1, 1 + zc), slice(1, 1 + D))
                w = slice(0, zc)
                I = {k: t[:, w, :] for k, t in inter.items()}
                fx, fy, fz = FX[:, w, :], FY[:, w, :], FZ[:, w, :]

                tt = nc.vector.tensor_tensor
                ts = nc.vector.tensor_scalar
                gp = nc.gpsimd.tensor_tensor
                act = nc.scalar.activation

                # geometry: fp32
                tt(I["dX"], npos[0][nsl], mpos[0](v)[msl], A.subtract)
                tt(I["dY"], npos[1][nsl], mpos[1](v)[msl], A.subtract)
                tt(I["dZ"], npos[2][nsl], mpos[2](v)[msl], A.subtract)
                act(I["QA"], I["dX"], AF.Square)
                act(I["QB"], I["dY"], AF.Square)
                act(I["QC"], I["dZ"], AF.Square)
                act(I["dXb"], I["dX"], AF.Copy)
                act(I["dYb"], I["dY"], AF.Copy)
                act(I["dZb"], I["dZ"], AF.Copy)
                tt(I["S"], I["QA"], I["QB"], A.add)
                tt(I["S"], I["S"], I["QC"], A.add)
                act(I["LN"], I["S"], AF.Ln, bias=b_eps[:], scale=1.0)
                act(I["R"], I["LN"], AF.Exp, bias=0.0, scale=-0.5)
                act(I["R2"], I["LN"], AF.Exp, bias=b_lneta[:], scale=-1.0)
                # spring term: KN - KN*PS*r  (ACT affine)
                act(I["Aa"], I["R"], AF.Identity, bias=b_kn[:], scale=-KN * PS)
                # contact gate on s directly: [dist < PS] == [s < PS^2]
                ts(I["G"], I["S"], PS * PS, None, A.is_lt)
                # damping: bf16
                tt(I["VA"], nvelb[0][nsl], mvelb[0](v)[msl], A.subtract)
                tt(I["VB"], nvelb[1][nsl], mvelb[1](v)[msl], A.subtract)
                tt(I["VC"], nvelb[2][nsl], mvelb[2](v)[msl], A.subtract)
                tt(I["P1"], I["VA"], I["dXb"], A.mult)
                tt(I["P2"], I["VB"], I["dYb"], A.mult)
                tt(I["TX"], I["VC"], I["dZb"], A.mult)
                tt(I["P1"], I["P1"], I["P2"], A.add)
                tt(I["P1"], I["P1"], I["TX"], A.add)
                tt(I["P2"], I["P1"], I["R2"], A.mult)   # ETA * dvn_raw / s
                tt(I["P3"], I["Aa"], I["P2"], A.add)
                tt(I["C"], I["P3"], I["G"], A.mult)     # gated coef (bf16 out)
                tt(I["TX"], I["C"], I["dXb"], A.mult)
                tt(I["TY"], I["C"], I["dYb"], A.mult)
                tt(I["TZ"], I["C"], I["dZb"], A.mult)
                tt(fx, fx, I["TX"], A.add)
                tt(fy, fy, I["TY"], A.add)
                tt(fz, fz, I["TZ"], A.add)

            def boundary_and_update(zc, nsrc, write_bf16):
                tt = nc.vector.tensor_tensor
                ts = nc.vector.tensor_scalar
                w = slice(0, zc)
                mkw = mk[:, w, 1:1 + D]
                # fp32 scratch only (the bf16 combo tags stay out of here)
                Aa = inter["QA"][:, w, :]
                G = inter["QB"][:, w, :]
                P = inter["QC"][:, w, :]
                T = inter["dX"][:, w, :]
                C = inter["dY"][:, w, :]
                S = inter["S"][:, w, :]
                for comp, (FF, grav) in enumerate(
                        [(FX, 0.0), (FY, 0.0), (FZ, GRAV)]):
                    p = nsrc[comp][:, 1:1 + zc, 1:1 + D]
                    vv = nsrc[3 + comp][:, 1:1 + zc, 1:1 + D]
                    f = FF[:, w, :]
                    ts(Aa, p, PS, None, A.is_gt)
                    ts(G, p, BLO_HI, None, A.is_lt)
                    tt(Aa, Aa, G, A.mult)            # lo
                    ts(G, p, BHI_TH, None, A.is_gt)  # hi
                    ts(T, p, -KN, KN * BLO_HI, A.mult, A.add)
                    tt(T, T, Aa, A.mult)
                    ts(C, p, -KN, KN * BHI_TH, A.mult, A.add)
                    tt(C, C, G, A.mult)
                    tt(T, T, C, A.add)
                    tt(Aa, Aa, G, A.add)             # lo + hi
                    tt(P, vv, Aa, A.mult)
                    ts(P, P, -ETA, None, A.mult)
                    tt(T, T, P, A.add)               # fb
                    tt(S, T, f, A.subtract)
                    if grav != 0.0:
                        ts(S, S, 1.0, grav, A.mult, A.add)
                    tt(S, S, mkw, A.mult)
                    ts(S, S, DT / PM, None, A.mult)
                    vn = VN[comp][:, w, 1:1 + D]
                    tt(vn, vv, S, A.add)
                    if write_bf16:
                        vnb = VNB[comp][:, w, 1:1 + D]
                        tt(vnb, vv, S, A.add)

            def phase(n, chunks, m_list, vel_src):
                """vel_src[m] -> ('ext', layer) or ('scr',) for bf16 vel loads"""
                for (w0, zc) in chunks:
                    zlo, zhi = w0 - 1, w0 + zc + 1
                    for f in range(3):
                        load(nt[f][:, 0:zc + 2, :],
                             ext[POS_NAMES[f]][n, zlo:zhi, 1:1 + D, :])
                        load(nt[3 + f][:, 0:zc + 2, :],
                             ext[VEL_NAMES[f]][n, zlo:zhi, 1:1 + D, :])
                        load(ntb[f][:, 0:zc + 2, :],
                             extb[VEL_NAMES[f]][n, zlo:zhi, 1:1 + D, :])
                    load(mk[:, 0:zc, :], ext["mk"][n, w0:w0 + zc, 1:1 + D, :])
                    nc.gpsimd.memset(FX[:, 0:zc, :], 0.0)
                    nc.gpsimd.memset(FY[:, 0:zc, :], 0.0)
                    nc.gpsimd.memset(FZ[:, 0:zc, :], 0.0)

                    npos = [nt[0], nt[1], nt[2]]
                    nvelb = ntb
                    for m in m_list:
                        same = (m == n)
                        for f in range(3):
                            vs = ((-1, 1) if same else (-1, 0, 1))
                            for v in vs:
                                load(mp[(f, v)][:, 0:zc + 2, :],
                                     ext[POS_NAMES[f]][m, zlo:zhi, 1 + v:1 + v + D, :])
                            src = vel_src[m]
                            for v in (-1, 0, 1):
                                if src[0] == "ext":
                                    load(mv[(f, v)][:, 0:zc + 2, :],
                                         extb[VEL_NAMES[f]][m, zlo:zhi,
                                                            1 + v:1 + v + D, :])
                                else:
                                    load(mv[(f, v)][:, 0:zc + 2, :],
                                         scr[f, w0 - 2:w0 + zc,
                                             1 + v:1 + v + D, :])
                        if same:
                            mpos = [(lambda f_: (lambda v: nt[f_] if v == 0
                                                 else mp[(f_, v)]))(f)
                                    for f in range(3)]
                        else:
                            mpos = [(lambda f_: (lambda v: mp[(f_, v)]))(f)
                                    for f in range(3)]
                        mvelb = [(lambda f_: (lambda v: mv[(f_, v)]))(f)
                                 for f in range(3)]
                        for sh in SHIFTS:
                            if same and sh == (0, 0, 0):
                                continue
                            emit_combo(zc, sh, npos, nvelb, mpos, mvelb)

                    for i in range(3):
                        nc.gpsimd.memset(VN[i][:, 0:zc, :], 0.0)
                        if n == 0:
                            nc.gpsimd.memset(VNB[i][:, 0:zc, :], 0.0)
                    boundary_and_update(zc, nt, write_bf16=(n == 0))
                    if n == 0:
                        for i in range(3):
                            nc.sync.dma_start(
                                scr[i, w0 - 1:w0 - 1 + zc, 1:1 + D, :]
                                .rearrange("z y x -> y z x"),
                                VNB[i][:, 0:zc, :])
                        olo, ohi = max(w0, 2), min(w0 + zc, 2 + ZP)
                        if ohi > olo:
                            for i in range(3):
                                nc.sync.dma_start(
                                    out_ext[i, olo - 2:ohi - 2, :, :]
                                    .rearrange("z y x -> y z x"),
                                    VN[i][:, olo - w0:ohi - w0, 1:1 + D])
                    else:
                        for i in range(3):
                            nc.sync.dma_start(
                                out_ext[3 + i, w0 - 2:w0 - 2 + zc, :, :]
                                .rearrange("z y x -> y z x"),
                                VN[i][:, 0:zc, 1:1 + D])

            # phase 0: n=0; m=0 (same layer) then m=1, vel from ext bf16
            phase(0, [(1, 6), (7, 6), (13, 6)], [0, 1],
                  {0: ("ext",), 1: ("ext",)})
            # phase 1: n=1; m=1 (same layer) then m=0 with vel from scratch
            phase(1, [(2, 6), (8, 6), (14, 4)], [1, 0],
                  {1: ("ext",), 0: ("scr",)})

    nc.compile()
    return nc


def _get_compiled():
    global _compiled
    if _compiled is None:
        _compiled = _build()
    return _compiled


def _pad_field(a, val):
    a = np.ascontiguousarray(a.reshape(2, D, D, D), dtype=np.float32)
    return np.pad(a, ((0, 0), (2, 2), (1, 1), (1, 1)), constant_values=val)


def _make_in_maps(inputs):
    import ml_dtypes

    padded = {
        "xg": _pad_field(inputs["x_grid"], SENT),
        "yg": _pad_field(inputs["y_grid"], SENT),
        "zg": _pad_field(inputs["z_grid"], SENT),
        "vx": _pad_field(inputs["vx_grid"], 0.0),
        "vy": _pad_field(inputs["vy_grid"], 0.0),
        "vz": _pad_field(inputs["vz_grid"], 0.0),
        "mk": _pad_field(inputs["mask"], 0.0),
    }
    for f in VEL_NAMES:
        padded[f + "b"] = padded[f].astype(ml_dtypes.bfloat16)

    in_maps = []
    for c in range(NCORES):
        z0 = ZP * c
        in_maps.append({k: np.ascontiguousarray(v[:, z0:z0 + NZIN])
                        for k, v in padded.items()})
    return in_maps


def kernel(x_grid, y_grid, z_grid, vx_grid, vy_grid, vz_grid, mask):
    from concourse.bass_utils import run_bass_kernel_spmd

    nc = _get_compiled()
    in_maps = _make_in_maps({
        "x_grid": x_grid, "y_grid": y_grid, "z_grid": z_grid,
        "vx_grid": vx_grid, "vy_grid": vy_grid, "vz_grid": vz_grid,
        "mask": mask,
    })
    res = run_bass_kernel_spmd(nc, in_maps, core_ids=list(range(NCORES)))

    out = np.empty((3, 2, 1, 1, D, D, D), np.float32)
    for c in range(NCORES):
        o = res.results[c]["out"]
        z0 = ZP * c
        for comp in range(3):
            out[comp, 0, 0, 0, z0:z0 + ZP] = o[comp]
            out[comp, 1, 0, 0, z0:z0 + ZP] = o[3 + comp]
    return out



# revision 8
# speedup vs baseline: 1.5181x; 1.5181x over previous
"""AI4DEM contact-force kernel for 8 TRN2 NeuronCores — v2.

Same physics/stencil reduction as v1 (125->27-point stencil, halo-2
z-sharding, sentinel-padded boundaries, two sequential layer phases with
layer-0's updated velocities feeding layer-1's cross-layer damping).

v1 was DVE-bound at 98.7% occupancy (4.50 ms).  v2 restructures to cut
DVE work:

* Force accumulation moved to the idle Tensor engine: per-combo force
  products T = coef*d stream through a bf16 identity matmul into PSUM
  (fp32 accumulate), removing the 3 fp32 DVE accumulate ops per combo
  and all force memsets.
* Newton's 3rd law for same-layer pairs: each of the 26 same-layer
  shifts pairs with its negation; the pair is computed once and the
  partner's force is scattered with a NEGATED, y-SHIFTED identity
  matmul (z/x shifts are free-dim window offsets).  26 combos -> 13
  pairs (+9 single-plane edge combos at the range top for the shifts
  whose scatter partner lies one core over).
* Custom fused DVE ops (registered per-NEFF at import): squared-distance
  chain (2 ops instead of 2 adds + 3 ACT squares), gated spring/damping
  coefficient factors select(s<PS^2, 1-PS*r, 0) and select(s<PS^2, r*r, 0)
  (folding the contact gate, removing the separate gate+mult and the
  ACT exp/affine), fused boundary-force and velocity-update chains.
* The whole force chain is rescaled by 1/KN (bf16 velocity copies are
  pre-scaled by ETA/KN on the host) so the fused ops fit the 2-scalar
  limit of the 2-free-dim custom-DVE encoding.

Geometry (position differences, distance, contact gate) stays fp32 so
the gate matches the reference bit-for-bit; the damping dot product and
force products run in bf16 (DVE 2x); accumulation is fp32 in PSUM.
"""

import math
import sys

import numpy as np

sys.path.insert(0, "/opt/trn_rl_repo")

D = 128
CELL = 0.1
PS = 0.1
KN = 6.0e6
_ALPHA = -math.log(0.5) / math.pi
_GAMMA = _ALPHA / math.sqrt(_ALPHA**2 + 1.0)
PM = 4.0 / 3.0 * 3.1415 * CELL**3 * 2700.0
ETA = 2.0 * _GAMMA * math.sqrt(KN * PM)
DT = 1e-4
SENT = 1.0e3
NCORES = 8
ZP = D // NCORES          # 16 owned planes per core
NZIN = ZP + 4             # input z extent (halo 2)
NY = D + 2
NX = D + 2

EK = ETA / KN             # bf16 velocity prescale
DTK = DT * KN / PM        # update scale (forces carried as F/KN)
GRAVK = -9.8 * PM / KN
PS2 = PS * PS
BLO_HI = 1.5 * PS
BHI_TH = D * CELL - 0.5 * PS - CELL

POS_NAMES = ["xg", "yg", "zg"]
VEL_NAMES = ["vx", "vy", "vz"]

# same-layer shift pair representatives (lexicographically positive half)
REPS = ([(1, sy, sx) for sy in (-1, 0, 1) for sx in (-1, 0, 1)]
        + [(0, 1, sx) for sx in (-1, 0, 1)] + [(0, 0, 1)])
# top-plane fixups: anti-reps with sz=-1 whose scatter source is off-range
EDGES = [(-1, sy, sx) for sy in (-1, 0, 1) for sx in (-1, 0, 1)]
CROSS = [(sz, sy, sx) for sz in (-1, 0, 1) for sy in (-1, 0, 1)
         for sx in (-1, 0, 1)]

_compiled = None
_ops = None


def _register_ops():
    """Register the fused DVE ops per-NEFF (runtime append to dve_ops.OPS)."""
    global _ops
    if _ops is not None:
        return _ops
    from concourse import dve_ops
    from concourse.dve_spec import (
        Spec, Src0, Src1, C0, C1, Zero, One, sq, select, lower)
    from concourse.dve_ops import has_src1
    from concourse.dve_uop import DveOpSpec

    def reg(name, body, reference):
        for o in dve_ops.OPS:
            if o.name == name:
                return o
        spec = Spec(body=body, reference=reference)
        op = dve_ops.DveOp(name, spec, subdim=False, uops_sha={})
        dve_ops.OPS.append(op)
        dve_ops.CUSTOM_DVE_SPECS[name] = spec
        dve_ops._SUB_OPCODE_FOR_NAME[name] = (
            dve_ops._CUSTOM_DVE_ROW_BASE + len(dve_ops.OPS) - 1)
        for ver in ("v3", "v4"):
            s = DveOpSpec(name=name, opcode=dve_ops.get_dve_sub_opcode(name),
                          uops=lower(spec, ver=ver), rd1_en=has_src1(spec))
            op.uops_sha[ver] = s.sha(ver)
        return op

    _ops = {
        # s1 = dx^2 + dy^2
        "SQ1": reg("AI4_SQ1", sq(Src0) + sq(Src1),
                   lambda in0, in1, c0, c1, c2: in0 * in0 + in1 * in1),
        # s = s1 + dz^2 + eps
        "SQ2": reg("AI4_SQ2", Src0 + sq(Src1) + C0,
                   lambda in0, in1, c0, c1, c2: in0 + in1 * in1 + c0),
        # gated spring factor (1 - PS*r), r = 1/dist ; C0=PS^2, C1=-PS
        "AAG": reg("AI4_AAG", select(Src0 < C0, One + C1 * Src1, Zero),
                   lambda in0, in1, c0, c1, c2:
                   np.where(in0 < c0, 1.0 + c1 * in1, 0.0)),
        # gated 1/s ; C0=PS^2
        "R2G": reg("AI4_R2G", select(Src0 < C0, sq(Src1), Zero),
                   lambda in0, in1, c0, c1, c2:
                   np.where(in0 < c0, in1 * in1, 0.0)),
        # boundary low shell: lo*( (C1-p) - vb ) ; C0=PS, C1=1.5PS, vb=v*EK
        "BLO": reg("AI4_BLO",
                   select((Src0 > C0) & (Src0 < C1), (C1 - Src0) - Src1, Zero),
                   lambda in0, in1, c0, c1, c2:
                   np.where((in0 > c0) & (in0 < c1), (c1 - in0) - in1, 0.0)),
        # boundary high shell: hi*( (C0-p) - vb ) ; C0=BHI_TH
        "BHI": reg("AI4_BHI", select(Src0 > C0, (C0 - Src0) - Src1, Zero),
                   lambda in0, in1, c0, c1, c2:
                   np.where(in0 > c0, (c0 - in0) - in1, 0.0)),
        # update: (f + C0) * C1 * mask
        "UPD": reg("AI4_UPD", (Src0 + C0) * C1 * Src1,
                   lambda in0, in1, c0, c1, c2: (in0 + c0) * c1 * in1),
    }
    return _ops


def _build():
    from contextlib import ExitStack
    from concourse import bacc, tile, mybir

    ops = _register_ops()
    f32 = mybir.dt.float32
    bf16 = mybir.dt.bfloat16
    A = mybir.AluOpType
    AF = mybir.ActivationFunctionType

    nc = bacc.Bacc("TRN2", target_bir_lowering=False, debug=False)

    # single activation-table set (Ln/Exp/Copy all live in
    # natural_log_exp_and_others; blank the rest so one load serves all).
    from concourse import hw_specs
    tabs = hw_specs.get_activation_tables(nc.m.arch)
    for k in tabs:
        if k != "natural_log_exp_and_others":
            tabs[k] = set()

    ext = {}
    for f in POS_NAMES:
        ext[f] = nc.dram_tensor(f, [2, NZIN, NY, NX], f32,
                                kind="ExternalInput").ap()
    for f in VEL_NAMES:
        ext[f] = nc.dram_tensor(f, [2, NZIN, NY, NX], f32,
                                kind="ExternalInput").ap()
    extb = {}
    for f in VEL_NAMES:
        extb[f] = nc.dram_tensor(f + "b", [2, NZIN, NY, NX], bf16,
                                 kind="ExternalInput").ap()
    ext["mk"] = nc.dram_tensor("mk", [2, NZIN, NY, NX], f32,
                               kind="ExternalInput").ap()
    out_ext = nc.dram_tensor("out", [6, ZP, D, D], f32,
                             kind="ExternalOutput").ap()
    # layer-0 updated velocities, EK-prescaled bf16, z local [-1,17)
    scr = nc.dram_tensor("v0s", [3, ZP + 2, NY, NX], bf16).ap()

    with tile.TileContext(nc) as tc:
        with ExitStack() as ctx:
            pool = ctx.enter_context(tc.tile_pool(name="sbuf", bufs=1))
            ppool = ctx.enter_context(tc.tile_pool(name="psum", bufs=1,
                                                   space="PSUM"))

            def mk(name, shape, dtp, bufs=1):
                return pool.tile(shape, dtp, name=name, tag=name, bufs=bufs)

            # field tiles
            npos = [mk(f"npos{f}", [D, 10, NX], f32) for f in range(3)]
            nvb = [mk(f"nvb{f}", [D, 10, NX], bf16) for f in range(3)]
            nvel = [mk(f"nvel{f}", [D, 8, D], f32) for f in range(3)]
            mkt = mk("mkt", [D, 8, D], f32)
            mpos = {(f, v): mk(f"mpos{f}_{v}", [D, 10, NX], f32)
                    for f in range(3) for v in (-1, 0, 1)}
            mvb = {(f, v): mk(f"mvb{f}_{v}", [D, 10, NX], bf16)
                   for f in range(3) for v in (-1, 0, 1)}

            # per-unit scratch: allocated per use so multi-buffer tags rotate
            _SCRATCH_BUFS = {"dx0": 2, "dx1": 2, "dx2": 2, "S": 2, "R": 1,
                             "dB0": 2, "dB1": 2, "dB2": 2, "aag": 1,
                             "r2g": 1, "va": 1, "m1": 1, "m2": 1,
                             "T0": 2, "T1": 2, "T2": 2}

            def sc(tag, dtp):
                return pool.tile([D, 8, D], dtp, name=tag, tag=tag,
                                 bufs=_SCRATCH_BUFS[tag])

            vn = [mk(f"vn{f}", [D, 8, D], f32) for f in range(3)]
            vnb = [mk(f"vnb{f}", [D, 8, NX], bf16) for f in range(3)]

            ZT = mk("ZT", [D, 8, D], bf16)
            zty = mk("zty", [D, NX], bf16)
            ident = mk("ident", [D, D], bf16)
            nident = {v: mk(f"nident{v}", [D, D], bf16) for v in (-1, 0, 1)}

            F = [ppool.tile([D, 8, D], f32, name=f"F{c}", tag=f"F{c}")
                 for c in range(3)]
            carry = [ppool.tile([D, 3, D], f32, name=f"carry{c}",
                                tag=f"carry{c}") for c in range(2)]

            # one-time init
            nc.gpsimd.memset(ZT[:], 0.0)
            nc.gpsimd.memset(zty[:], 0.0)
            for c in range(3):
                nc.gpsimd.memset(vnb[c][:], 0.0)
            nc.gpsimd.memset(ident[:], 0.0)
            nc.gpsimd.affine_select(
                out=ident[:], in_=ident[:], compare_op=A.not_equal,
                fill=1.0, base=0, pattern=[[-1, D]], channel_multiplier=1)
            for v in (-1, 0, 1):
                nc.gpsimd.memset(nident[v][:], 0.0)
                # out[m] -= rhs[m + v]  <=>  lhsT[k, m] = -1 at m = k - v
                nc.gpsimd.affine_select(
                    out=nident[v][:], in_=nident[v][:], compare_op=A.not_equal,
                    fill=-1.0, base=-v, pattern=[[-1, D]],
                    channel_multiplier=1)
            for f in range(3):
                nc.sync.dma_start(scr[f, :, 0, :], zty[0:ZP + 2, :])
                nc.sync.dma_start(scr[f, :, NY - 1, :], zty[0:ZP + 2, :])

            tt = nc.vector.tensor_tensor
            act = nc.scalar.activation
            cd = nc.vector._custom_dve
            mm = nc.tensor.matmul

            def load(tile_t, src_ap):
                nc.sync.dma_start(tile_t, src_ap.rearrange("z y x -> y z x"))

            def bank_splits(lo, hi):
                out = []
                a = lo
                while a < hi:
                    b = min(hi, (a // 4 + 1) * 4)
                    out.append((a, b))
                    a = b
                return out

            def zero_group(zc, start, stop, with_carry):
                for c in range(3):
                    for (a, b) in bank_splits(0, zc):
                        mm(F[c][:, a:b, :], ident[:], ZT[:, a:b, :],
                           start=start, stop=stop, skip_group_check=True)
                if with_carry is not None:
                    for c in range(3):
                        mm(carry[with_carry][:, c:c + 1, :], ident[:],
                           ZT[:, 0:1, :],
                           start=start, stop=stop, skip_group_check=True)

            def emit_unit(s, zcu, nz0, fz0, zc, pget, vget, scatter, cout):
                """One gather (or pair) unit for shift s.

                pget(f, v) / vget(f, v): m-side position / bf16-velocity
                tile for y-variant v; v=None means the n-side tile.
                """
                sz, sy, sx = s
                v = -sy
                nzs = slice(nz0, nz0 + zcu)
                mzs = slice(nz0 - sz, nz0 - sz + zcu)
                nxs = slice(1, 1 + D)
                mxs = slice(1 - sx, 1 - sx + D)
                w = slice(0, zcu)

                dXYZ = [sc(f"dx{f}", f32) for f in range(3)]
                S = sc("S", f32)
                R = sc("R", f32)
                dB = [sc(f"dB{f}", bf16) for f in range(3)]
                aag = sc("aag", bf16)
                r2g = sc("r2g", bf16)
                va = sc("va", bf16)
                m1 = sc("m1", bf16)
                m2 = sc("m2", bf16)
                T3 = [sc(f"T{f}", bf16) for f in range(3)]

                for f in range(3):
                    tt(dXYZ[f][:, w, :], npos[f][:, nzs, nxs],
                       pget(f, v)[:, mzs, mxs], A.subtract)
                cd(ops["SQ1"], out=S[:, w, :], in0=dXYZ[0][:, w, :],
                   in1=dXYZ[1][:, w, :])
                cd(ops["SQ2"], out=S[:, w, :], in0=S[:, w, :],
                   in1=dXYZ[2][:, w, :], s0=1e-8)
                act(R[:, w, :], S[:, w, :], AF.Ln)
                act(R[:, w, :], R[:, w, :], AF.Exp, bias=0.0, scale=-0.5)
                cd(ops["AAG"], out=aag[:, w, :], in0=S[:, w, :],
                   in1=R[:, w, :], s0=PS2, s1=-PS)
                cd(ops["R2G"], out=r2g[:, w, :], in0=S[:, w, :],
                   in1=R[:, w, :], s0=PS2)
                for f in range(3):
                    act(dB[f][:, w, :], dXYZ[f][:, w, :], AF.Copy)
                # damping dot (bf16, EK-prescaled velocities)
                tt(va[:, w, :], nvb[0][:, nzs, nxs], vget(0, v)[:, mzs, mxs],
                   A.subtract)
                tt(m1[:, w, :], va[:, w, :], dB[0][:, w, :], A.mult)
                tt(va[:, w, :], nvb[1][:, nzs, nxs], vget(1, v)[:, mzs, mxs],
                   A.subtract)
                tt(m2[:, w, :], va[:, w, :], dB[1][:, w, :], A.mult)
                tt(m1[:, w, :], m1[:, w, :], m2[:, w, :], A.add)
                tt(va[:, w, :], nvb[2][:, nzs, nxs], vget(2, v)[:, mzs, mxs],
                   A.subtract)
                tt(m2[:, w, :], va[:, w, :], dB[2][:, w, :], A.mult)
                tt(m1[:, w, :], m1[:, w, :], m2[:, w, :], A.add)
                tt(m2[:, w, :], m1[:, w, :], r2g[:, w, :], A.mult)
                tt(m1[:, w, :], m2[:, w, :], aag[:, w, :], A.add)  # coef/KN
                for f in range(3):
                    tt(T3[f][:, w, :], m1[:, w, :], dB[f][:, w, :], A.mult)

                # gather: F[fz] += T[fz - fz0]
                for c in range(3):
                    T = T3[c]
                    for (a, b) in bank_splits(fz0, fz0 + zcu):
                        mm(F[c][:, a:b, :], ident[:],
                           T[:, a - fz0:b - fz0, :],
                           start=False, stop=False, skip_group_check=True)
                    if scatter:
                        # F[p - s] -= T[p] : negated y-shifted identity
                        oxl = max(0, -sx)
                        oxh = D + min(0, -sx)
                        oxs = slice(oxl, oxh)
                        txs = slice(oxl + sx, oxh + sx)
                        lhs = nident[sy][:]
                        for (a, b) in bank_splits(max(0, -sz), zc - sz):
                            mm(F[c][:, a:b, oxs], lhs,
                               T[:, a + sz:b + sz, txs],
                               start=False, stop=False, skip_group_check=True)
                        if sz == 1 and cout is not None:
                            mm(carry[cout][:, c:c + 1, oxs], lhs,
                               T[:, 0:1, txs],
                               start=False, stop=False, skip_group_check=True)

            def do_pass(n, m_other, p0, zc, top, cin, cout, other_vget):
                """One z-pass of phase n: planes iz in [p0, p0+zc)."""
                zin = slice(p0 - 1, p0 + zc + 1)
                # n-side loads
                for f in range(3):
                    load(npos[f][:, 0:zc + 2, :],
                         ext[POS_NAMES[f]][n, zin, 1:1 + D, :])
                    load(nvb[f][:, 0:zc + 2, :],
                         extb[VEL_NAMES[f]][n, zin, 1:1 + D, :])
                    load(nvel[f][:, 0:zc, :],
                         ext[VEL_NAMES[f]][n, p0:p0 + zc, 1:1 + D, 1:1 + D])
                load(mkt[:, 0:zc, :],
                     ext["mk"][n, p0:p0 + zc, 1:1 + D, 1:1 + D])

                zero_group(zc, True, False, cout)

                # same-layer block (pairs + top-edge fixups)
                for f in range(3):
                    for v in (-1, 1):
                        load(mpos[(f, v)][:, 0:zc + 2, :],
                             ext[POS_NAMES[f]][n, zin, 1 + v:1 + v + D, :])
                        load(mvb[(f, v)][:, 0:zc + 2, :],
                             extb[VEL_NAMES[f]][n, zin, 1 + v:1 + v + D, :])

                def pget_same(f, v):
                    return npos[f] if v == 0 else mpos[(f, v)]

                def vget_same(f, v):
                    return nvb[f] if v == 0 else mvb[(f, v)]

                for s in REPS:
                    emit_unit(s, zc, 1, 0, zc, pget_same, vget_same,
                              scatter=True, cout=cout)
                if top:
                    for s in EDGES:
                        emit_unit(s, 1, zc, zc - 1, zc, pget_same, vget_same,
                                  scatter=False, cout=None)

                # cross-layer block
                for f in range(3):
                    for v in (-1, 0, 1):
                        load(mpos[(f, v)][:, 0:zc + 2, :],
                             ext[POS_NAMES[f]][m_other, zin,
                                               1 + v:1 + v + D, :])
                        other_vget(f, v, zin, p0, zc)

                def pget_x(f, v):
                    return mpos[(f, v)]

                def vget_x(f, v):
                    return mvb[(f, v)]

                for s in CROSS:
                    emit_unit(s, zc, 1, 0, zc, pget_x, vget_x,
                              scatter=False, cout=None)

                zero_group(zc, False, True, cout)

                # carry-in: F[top plane] += carry (via SBUF; PSUM+PSUM
                # reads are not allowed in one instruction)
                if cin is not None:
                    csb = pool.tile([D, 3, D], f32, name="csb", tag="csb")
                    nc.vector.tensor_copy(csb[:], carry[cin][:])
                    for c in range(3):
                        tt(F[c][:, zc - 1, :], F[c][:, zc - 1, :],
                           csb[:, c, :], A.add)

                # boundary + update per component (force chain is F/KN)
                w = slice(0, zc)
                for c in range(3):
                    p_w = npos[c][:, 1:1 + zc, 1:1 + D]
                    vb_w = nvb[c][:, 1:1 + zc, 1:1 + D]
                    fb = sc("S", f32)[:, w, :]
                    fb2 = sc("R", f32)[:, w, :]
                    cd(ops["BLO"], out=fb, in0=p_w, in1=vb_w, s0=PS,
                       s1=BLO_HI)
                    cd(ops["BHI"], out=fb2, in0=p_w, in1=vb_w, s0=BHI_TH)
                    tt(fb, fb, fb2, A.add)
                    tt(fb, fb, F[c][:, w, :], A.subtract)   # fb - F/KN
                    cd(ops["UPD"], out=fb, in0=fb, in1=mkt[:, w, :],
                       s0=(GRAVK if c == 2 else 0.0), s1=DTK)
                    tt(vn[c][:, w, :], nvel[c][:, w, :], fb, A.add)

                # emit outputs
                olo = max(p0, 2)
                ohi = min(p0 + zc, 2 + ZP)
                if n == 0:
                    for c in range(3):
                        act(vnb[c][:, w, 1:1 + D], vn[c][:, w, :], AF.Copy,
                            bias=0.0, scale=EK)
                        nc.sync.dma_start(
                            scr[c, p0 - 1:p0 - 1 + zc, 1:1 + D, :]
                            .rearrange("z y x -> y z x"),
                            vnb[c][:, 0:zc, :])
                        if ohi > olo:
                            nc.sync.dma_start(
                                out_ext[c, olo - 2:ohi - 2, :, :]
                                .rearrange("z y x -> y z x"),
                                vn[c][:, olo - p0:ohi - p0, :])
                else:
                    for c in range(3):
                        nc.sync.dma_start(
                            out_ext[3 + c, p0 - 2:p0 - 2 + zc, :, :]
                            .rearrange("z y x -> y z x"),
                            vn[c][:, 0:zc, :])

            # ---- phase 0: n=0, m_other=1 (velocities from extb) ----
            def vload_ext1(f, v, zin, p0, zc):
                load(mvb[(f, v)][:, 0:zc + 2, :],
                     extb[VEL_NAMES[f]][1, zin, 1 + v:1 + v + D, :])

            do_pass(0, 1, 13, 6, True, None, 0, vload_ext1)
            do_pass(0, 1, 7, 6, False, 0, 1, vload_ext1)
            do_pass(0, 1, 1, 6, False, 1, None, vload_ext1)

            # ---- phase 1: n=1, m_other=0 (velocities from scr) ----
            def vload_scr(f, v, zin, p0, zc):
                load(mvb[(f, v)][:, 0:zc + 2, :],
                     scr[f, p0 - 2:p0 + zc, 1 + v:1 + v + D, :])

            do_pass(1, 0, 10, 8, True, None, 0, vload_scr)
            do_pass(1, 0, 2, 8, False, 0, None, vload_scr)

    nc.compile()
    return nc


def _get_compiled():
    global _compiled
    if _compiled is None:
        _compiled = _build()
    return _compiled


def _pad_field(a, val):
    a = np.ascontiguousarray(a.reshape(2, D, D, D), dtype=np.float32)
    return np.pad(a, ((0, 0), (2, 2), (1, 1), (1, 1)), constant_values=val)


def _make_in_maps(inputs):
    import ml_dtypes

    padded = {
        "xg": _pad_field(inputs["x_grid"], SENT),
        "yg": _pad_field(inputs["y_grid"], SENT),
        "zg": _pad_field(inputs["z_grid"], SENT),
        "vx": _pad_field(inputs["vx_grid"], 0.0),
        "vy": _pad_field(inputs["vy_grid"], 0.0),
        "vz": _pad_field(inputs["vz_grid"], 0.0),
        "mk": _pad_field(inputs["mask"], 0.0),
    }
    for f in VEL_NAMES:
        padded[f + "b"] = (padded[f] * EK).astype(ml_dtypes.bfloat16)

    in_maps = []
    for c in range(NCORES):
        z0 = ZP * c
        in_maps.append({k: np.ascontiguousarray(v[:, z0:z0 + NZIN])
                        for k, v in padded.items()})
    return in_maps


def kernel(x_grid, y_grid, z_grid, vx_grid, vy_grid, vz_grid, mask):
    from concourse.bass_utils import run_bass_kernel_spmd

    nc = _get_compiled()
    in_maps = _make_in_maps({
        "x_grid": x_grid, "y_grid": y_grid, "z_grid": z_grid,
        "vx_grid": vx_grid, "vy_grid": vy_grid, "vz_grid": vz_grid,
        "mask": mask,
    })
    res = run_bass_kernel_spmd(nc, in_maps, core_ids=list(range(NCORES)))

    out = np.empty((3, 2, 1, 1, D, D, D), np.float32)
    for c in range(NCORES):
        o = res.results[c]["out"]
        z0 = ZP * c
        for comp in range(3):
            out[comp, 0, 0, 0, z0:z0 + ZP] = o[comp]
            out[comp, 1, 0, 0, z0:z0 + ZP] = o[3 + comp]
    return out
